# revision 23
# baseline (speedup 1.0000x reference)
"""LocalWindowAttention (block-causal) Trainium2 kernel, 8 NeuronCores.

Sharding: tensor-parallel over heads. Core c owns head-columns
[c*128, (c+1)*128) of the D=1024 hidden dim (2 heads x head_dim 64):
  - computes Q/K/V projections for its head slice (transposed layout),
  - block-causal attention for its 2 heads,
  - AllGathers its normalized attention outputs (two chunk-pair AGs),
  - applies the full Wo to the gathered activations for its 128 output
    rows of final^T [1024, 2048]. Host reassembles.

v3:
  - bf16 operands on-chip (PSUM stays f32): 1 cycle/row at any free
    dim, half the DMA of f32.
  - x DMAs are emitted interleaved with the first projection pass's
    matmuls, round-robin over four queues, so each matmul's DMA wait
    only covers chunks emitted before it -> PE starts ~2.5us after the
    queues open instead of after the full x load.
  - tiny warmup AllGather absorbs the CC engine's ~45us cold start.
  - attention runs as one flat (chunk, keyblock) stream with the
    score/exp pipeline carried across chunk boundaries (no drain
    stalls); normalize is emitted inline after each chunk's last AV.
  - two AllGathers (chunks {0,1} after c1, {2,3} after c3); the
    {0,1} output projection covers the second AG's latency.
"""

import numpy as np
import ml_dtypes

import concourse.bacc as bacc
import concourse.tile as tile
from concourse import mybir
from concourse.bass_utils import run_bass_kernel_spmd
from concourse.masks import make_identity

B, T, D = 1, 2048, 1024
H, HD, W = 16, 64, 128
N_CORES = 8
HS = D // N_CORES        # 128 head-columns per core (2 heads)
HPC = H // N_CORES       # heads per core
QW = 512                 # query-chunk width (free dim of S^T tiles)
NQ = T // QW             # 4 query chunks
NK = T // W              # 16 key chunks of 128
ND = D // 128            # 8 contraction chunks over D
SCALE = HD ** -0.5

F32 = mybir.dt.float32
BF16 = mybir.dt.bfloat16
Exp = mybir.ActivationFunctionType.Exp

_compiled = {}


def _build():
    nc = bacc.Bacc("TRN2", target_bir_lowering=False, debug=False,
                   num_devices=N_CORES)
    xT_ap = nc.dram_tensor("xT", [D, T], BF16, kind="ExternalInput").ap()
    wq_ap = nc.dram_tensor("wq", [D, HS], BF16, kind="ExternalInput").ap()
    wk_ap = nc.dram_tensor("wk", [D, HS], BF16, kind="ExternalInput").ap()
    wv_ap = nc.dram_tensor("wv", [D, HS], BF16, kind="ExternalInput").ap()
    wo_ap = nc.dram_tensor("wo", [D, HS], BF16, kind="ExternalInput").ap()
    y_ap = nc.dram_tensor("y", [HS, T], F32, kind="ExternalOutput").ap()

    with tile.TileContext(nc) as tc:
        _body(tc, xT_ap, wq_ap, wk_ap, wv_ap, wo_ap, y_ap)
    nc.compile()
    return nc


def _body(tc, xT_ap, wq_ap, wk_ap, wv_ap, wo_ap, y_ap):
    nc = tc.nc
    from contextlib import ExitStack
    with ExitStack() as ctx:
        singles = ctx.enter_context(tc.tile_pool(name="singles", bufs=1))
        work = ctx.enter_context(tc.tile_pool(name="work", bufs=4))
        es_pool = ctx.enter_context(tc.tile_pool(name="es_pool", bufs=8))
        dram = ctx.enter_context(tc.tile_pool(name="dram", bufs=1, space="DRAM"))

        # ---- identity (for V transposes) then warmup collective ----------
        ident_f32 = singles.tile([128, 128], F32, tag="ident_f32")
        make_identity(nc, ident_f32)
        ident = singles.tile([128, 128], BF16, tag="ident")
        nc.vector.tensor_copy(ident[:], ident_f32[:])

        warm_in = dram.tile([128, 8], F32, name="warm_in")
        warm_out = dram.tile([N_CORES, 128, 8], F32, addr_space="Shared",
                             name="warm_out")
        nc.gpsimd.collective_compute(
            "AllGather", mybir.AluOpType.bypass,
            replica_groups=[list(range(N_CORES))],
            ins=[warm_in.opt()], outs=[warm_out.opt()])

        # ---- weight DMAs, then x chunks interleaved with pass-A matmuls --
        wq = singles.tile([128, ND, HS], BF16, tag="wq")
        wk = singles.tile([128, ND, HS], BF16, tag="wk")
        wv = singles.tile([128, ND, HS], BF16, tag="wv")
        wo = singles.tile([128, ND, HS], BF16, tag="wo")
        nc.sync.dma_start(out=wq[:], in_=wq_ap.rearrange("(c p) m -> p c m", p=128))
        nc.scalar.dma_start(out=wk[:], in_=wk_ap.rearrange("(c p) m -> p c m", p=128))
        nc.gpsimd.dma_start(out=wv[:], in_=wv_ap.rearrange("(c p) m -> p c m", p=128))

        qT = singles.tile([128, T], BF16, tag="qT")
        kT = singles.tile([128, T], BF16, tag="kT")
        vT = singles.tile([128, T], BF16, tag="vT")
        vn = [singles.tile([128, NK, HD + 1], BF16, tag=f"vn{h}", name=f"vn{h}")
              for h in range(HPC)]
        outT = singles.tile([128, T], BF16, tag="outT")

        ones = singles.tile([128, 1], F32, tag="ones")
        nc.vector.memset(ones[:], 1.0)
        for h in range(HPC):
            nc.vector.tensor_copy(vn[h][:, :, HD:],
                                  ones[:].unsqueeze(1).to_broadcast([128, NK, 1]))

        qdma = [nc.sync, nc.scalar, nc.gpsimd]
        xts = []

        with tc.tile_pool(name="pp", bufs=1, space="PSUM") as pp, \
             tc.tile_pool(name="pt", bufs=2, space="PSUM") as pt:
            # pass A (cols 0..1023): emit each x-chunk DMA right before the
            # matmuls that consume it so the DMA waits stay minimal
            ps_q = pp.tile([128, 2, QW], F32, tag="q", name="ps_q")
            ps_k = pp.tile([128, 2, QW], F32, tag="k", name="ps_k")
            ps_v = pp.tile([128, 2, QW], F32, tag="v", name="ps_v")
            # x-chunk -> queue (x0 split across sync+scalar). x1 goes to
            # sync, not gpsimd: behind wv it would arrive after the d=1
            # matmuls want it (~4us PE stall)
            XQ = [None, 0, 1, 2, 0, 1, 2, 0]
            for d in range(ND):
                xt = singles.tile([128, T], BF16, tag=f"x{d}", name=f"xt{d}")
                if d == 0:
                    # split the first chunk across two queues: the first
                    # matmuls wait on it, so halving its DMA time moves the
                    # whole schedule up
                    nc.sync.dma_start(out=xt[0:64, :], in_=xT_ap[0:64, :])
                    nc.scalar.dma_start(out=xt[64:128, :], in_=xT_ap[64:128, :])
                else:
                    qdma[XQ[d]].dma_start(out=xt[:],
                                          in_=xT_ap[d * 128:(d + 1) * 128, :])
                xts.append(xt)
                f = (d == 0)
                l = (d == ND - 1)
                for ps, w in ((ps_q, wq), (ps_k, wk), (ps_v, wv)):
                    for sub in range(2):
                        cs = slice(sub * QW, (sub + 1) * QW)
                        nc.tensor.matmul(ps[:, sub, :], w[:, d, :],
                                         xts[d][:, cs], start=f, stop=l)
            nc.gpsimd.dma_start(out=wo[:],
                                in_=wo_ap.rearrange("(c p) m -> p c m", p=128))
            nc.vector.tensor_copy(vT[:, 0:2 * QW], ps_v[:])
            nc.vector.tensor_copy(qT[:, 0:2 * QW], ps_q[:])
            nc.vector.tensor_copy(kT[:, 0:2 * QW], ps_k[:])
            # V transposes for blocks 0-7 (PE; waits only on the v copy)
            for tk in range(8):
                ps_t = pt.tile([128, W], BF16, tag="t", name="ps_t")
                nc.tensor.transpose(ps_t[:], vT[:, tk * W:(tk + 1) * W], ident[:])
                for h in range(HPC):
                    nc.vector.tensor_copy(vn[h][:, tk, :HD],
                                          ps_t[:, h * HD:(h + 1) * HD])

            # pass B (cols 1024..2047): all of x is resident by now
            ps_q = pp.tile([128, 2, QW], F32, tag="q", name="ps_qB")
            ps_k = pp.tile([128, 2, QW], F32, tag="k", name="ps_kB")
            ps_v = pp.tile([128, 2, QW], F32, tag="v", name="ps_vB")
            for d in range(ND):
                f = (d == 0)
                l = (d == ND - 1)
                for ps, w in ((ps_q, wq), (ps_k, wk), (ps_v, wv)):
                    for sub in range(2):
                        cs = slice(2 * QW + sub * QW, 2 * QW + (sub + 1) * QW)
                        nc.tensor.matmul(ps[:, sub, :], w[:, d, :],
                                         xts[d][:, cs], start=f, stop=l)
            nc.vector.tensor_copy(vT[:, 2 * QW:], ps_v[:])
            nc.vector.tensor_copy(qT[:, 2 * QW:], ps_q[:])
            nc.vector.tensor_copy(kT[:, 2 * QW:], ps_k[:])
            for tk in range(8, NK):
                ps_t = pt.tile([128, W], BF16, tag="t", name="ps_t")
                nc.tensor.transpose(ps_t[:], vT[:, tk * W:(tk + 1) * W], ident[:])
                for h in range(HPC):
                    nc.vector.tensor_copy(vn[h][:, tk, :HD],
                                          ps_t[:, h * HD:(h + 1) * HD])

        # ---- attention: flat (chunk, keyblock) stream, pipelined exps ----
        # AllGather groups (start_chunk, n_chunks). Each gather has a
        # large fixed cost (~15-20us) regardless of payload, so two equal
        # groups win: {c0,c1} triggers right after c1 (early, absorbing
        # the CC engine's variable init window) and {c2,c3} right after
        # c3; the {c0,c1} output projection covers the second gather.
        PAIRS = [(0, 2), (2, 2)]
        ag_in = [dram.tile([HS, n * QW], BF16, name=f"ag_in{j}")
                 for j, (s, n) in enumerate(PAIRS)]
        ag_out = [dram.tile([N_CORES, HS, n * QW], BF16, addr_space="Shared",
                            name=f"ag_out{j}") for j, (s, n) in enumerate(PAIRS)]

        gt_pool = ctx.enter_context(tc.tile_pool(name="gt_pool", bufs=1))
        gts = {}

        with tc.tile_pool(name="pa", bufs=2, space="PSUM") as pa, \
             tc.tile_pool(name="po", bufs=2, space="PSUM") as po:
            ps_os = {}

            def s_exp(t, tk):
                qs = max(0, (tk - 4 * t) * W)
                ps_s = pa.tile([128, 2 * QW], F32, tag="s", name="ps_s")
                for h in range(HPC):
                    hrows = slice(h * HD, (h + 1) * HD)
                    nc.tensor.matmul(
                        ps_s[:, h * QW + qs:(h + 1) * QW],
                        kT[hrows, tk * W:(tk + 1) * W],
                        qT[hrows, t * QW + qs:(t + 1) * QW],
                        start=True, stop=True)
                e = es_pool.tile([128, 2 * QW], BF16, tag="es", name="es")
                nc.scalar.activation(out=e[:, qs:], in_=ps_s[:, qs:],
                                     func=Exp, scale=SCALE)
                return e

            def av(t, tk, e):
                qs = max(0, (tk - 4 * t) * W)
                n_tk = 4 * t + 4
                if tk == 0:
                    ps_os[t] = po.tile([HD + 1, 2, QW], F32, tag="o",
                                       name="ps_o")
                ps_o = ps_os[t]
                for h in range(HPC):
                    nc.tensor.matmul(ps_o[:, h, qs:], vn[h][:, tk, :],
                                     e[:, h * QW + qs:(h + 1) * QW],
                                     start=(tk == 0), stop=(tk == n_tk - 1))
                if tk == n_tk - 1:
                    _normalize(t, ps_o)

            def _normalize(t, ps_o):
                cols = slice(t * QW, (t + 1) * QW)
                bc = work.tile([HD, 2, QW], F32, tag="bc", name="bc")
                for h in range(HPC):
                    rec = work.tile([1, QW], F32, tag=f"rec{h}", name="rec")
                    nc.vector.reciprocal(out=rec[:], in_=ps_o[HD:, h, :])
                    nc.gpsimd.partition_broadcast(bc[:, h, :], rec[:])
                for h in range(HPC):
                    hrows = slice(h * HD, (h + 1) * HD)
                    nc.vector.tensor_mul(outT[hrows, cols],
                                         ps_o[:HD, h, :], bc[:, h, :])
                jmatch = [j for j, (s, n) in enumerate(PAIRS)
                          if t == s + n - 1]
                if jmatch:
                    j = jmatch[0]
                    s, n = PAIRS[j]
                    pcols = slice(s * QW, (s + n) * QW)
                    nc.gpsimd.dma_start(out=ag_in[j][:], in_=outT[:, pcols])
                    nc.gpsimd.collective_compute(
                        "AllGather", mybir.AluOpType.bypass,
                        replica_groups=[list(range(N_CORES))],
                        ins=[ag_in[j].opt()], outs=[ag_out[j].opt()])
                    # prefetch the gathered slabs. DMA waits are cumulative
                    # per queue, so each group gets its own queue: j0 on
                    # sync (nothing later on sync until its own y store),
                    # j1 on scalar (all exps are done by then) -- the j0
                    # output projection must not wait behind j1's gather.
                    gq = nc.sync if j == 0 else nc.scalar
                    gts[j] = []
                    for c in range(N_CORES):
                        g = gt_pool.tile([128, n * QW], BF16,
                                         tag=f"g{j}_{c}", name=f"g{j}_{c}")
                        gq.dma_start(out=g[:], in_=ag_out[j][c])
                        gts[j].append(g)

            items = [(t, tk) for t in range(NQ) for tk in range(4 * t + 4)]
            pend = []
            for it in items:
                pend.append((it, s_exp(*it)))
                if len(pend) > 2:
                    (pt_, ptk), pe_ = pend.pop(0)
                    av(pt_, ptk, pe_)
            for (pt_, ptk), pe_ in pend:
                av(pt_, ptk, pe_)

        # ---- output projection on gathered activations -------------------
        with tc.tile_pool(name="py", bufs=1, space="PSUM") as py:
            for j, (s, n) in enumerate(PAIRS):
                ps_y = [py.tile([128, QW], F32, tag=f"y{i}", name=f"ps_y{i}")
                        for i in range(n)]
                for c in range(N_CORES):
                    g = gts[j][c]
                    for i in range(n):
                        nc.tensor.matmul(ps_y[i][:], wo[:, c, :],
                                         g[:, i * QW:(i + 1) * QW],
                                         start=(c == 0), stop=(c == N_CORES - 1))
                for i in range(n):
                    t = s + i
                    cols = slice(t * QW, (t + 1) * QW)
                    cy = work.tile([128, QW], F32, tag="cy")
                    nc.vector.tensor_copy(cy[:], ps_y[i][:])
                    nc.sync.dma_start(out=y_ap[:, cols], in_=cy[:])


def make_in_maps(x, Wq, Wk, Wv, Wo):
    bf = ml_dtypes.bfloat16
    xT = np.ascontiguousarray(np.asarray(x).reshape(T, D).T).astype(bf)
    in_maps = []
    for c in range(N_CORES):
        hs = slice(c * HS, (c + 1) * HS)
        in_maps.append({
            "xT": xT,
            "wq": np.ascontiguousarray(np.asarray(Wq)[:, hs]).astype(bf),
            "wk": np.ascontiguousarray(np.asarray(Wk)[:, hs]).astype(bf),
            "wv": np.ascontiguousarray(np.asarray(Wv)[:, hs]).astype(bf),
            "wo": np.ascontiguousarray(np.asarray(Wo)[:, hs]).astype(bf),
        })
    return in_maps


def kernel(x, Wq, Wk, Wv, Wo):
    if "nc" not in _compiled:
        _compiled["nc"] = _build()
    nc = _compiled["nc"]

    in_maps = make_in_maps(x, Wq, Wk, Wv, Wo)
    res = run_bass_kernel_spmd(nc, in_maps, list(range(N_CORES)))
    finalT = np.concatenate([res.results[c]["y"] for c in range(N_CORES)], axis=0)
    return np.ascontiguousarray(finalT.T).reshape(B, T, D)


# revision 25
# speedup vs baseline: 1.0202x; 1.0202x over previous
"""LocalWindowAttention (block-causal) Trainium2 kernel, 8 NeuronCores.

Sharding: tensor-parallel over heads. Core c owns head-columns
[c*128, (c+1)*128) of the D=1024 hidden dim (2 heads x head_dim 64):
  - computes Q/K/V projections for its head slice (transposed layout),
  - block-causal attention for its 2 heads,
  - AllGathers its normalized attention outputs (two chunk-pair AGs),
  - applies the full Wo to the gathered activations for its 128 output
    rows of final^T [1024, 2048]. Host reassembles.

v3:
  - bf16 operands on-chip (PSUM stays f32): 1 cycle/row at any free
    dim, half the DMA of f32.
  - x DMAs are emitted interleaved with the first projection pass's
    matmuls, round-robin over four queues, so each matmul's DMA wait
    only covers chunks emitted before it -> PE starts ~2.5us after the
    queues open instead of after the full x load.
  - tiny warmup AllGather absorbs the CC engine's ~45us cold start.
  - attention runs as one flat (chunk, keyblock) stream with the
    score/exp pipeline carried across chunk boundaries (no drain
    stalls); normalize is emitted inline after each chunk's last AV.
  - two AllGathers (chunks {0,1} after c1, {2,3} after c3); the
    {0,1} output projection covers the second AG's latency.
"""

import numpy as np
import ml_dtypes

import concourse.bacc as bacc
import concourse.tile as tile
from concourse import mybir
from concourse.bass_utils import run_bass_kernel_spmd
from concourse.masks import make_identity

B, T, D = 1, 2048, 1024
H, HD, W = 16, 64, 128
N_CORES = 8
HS = D // N_CORES        # 128 head-columns per core (2 heads)
HPC = H // N_CORES       # heads per core
QW = 512                 # query-chunk width (free dim of S^T tiles)
NQ = T // QW             # 4 query chunks
NK = T // W              # 16 key chunks of 128
ND = D // 128            # 8 contraction chunks over D
SCALE = HD ** -0.5

F32 = mybir.dt.float32
BF16 = mybir.dt.bfloat16
Exp = mybir.ActivationFunctionType.Exp

_compiled = {}


def _build():
    nc = bacc.Bacc("TRN2", target_bir_lowering=False, debug=False,
                   num_devices=N_CORES)
    xT_ap = nc.dram_tensor("xT", [D, T], BF16, kind="ExternalInput").ap()
    wq_ap = nc.dram_tensor("wq", [D, HS], BF16, kind="ExternalInput").ap()
    wk_ap = nc.dram_tensor("wk", [D, HS], BF16, kind="ExternalInput").ap()
    wv_ap = nc.dram_tensor("wv", [D, HS], BF16, kind="ExternalInput").ap()
    wo_ap = nc.dram_tensor("wo", [D, HS], BF16, kind="ExternalInput").ap()
    y_ap = nc.dram_tensor("y", [HS, T], F32, kind="ExternalOutput").ap()

    with tile.TileContext(nc) as tc:
        _body(tc, xT_ap, wq_ap, wk_ap, wv_ap, wo_ap, y_ap)
    nc.compile()
    return nc


def _body(tc, xT_ap, wq_ap, wk_ap, wv_ap, wo_ap, y_ap):
    nc = tc.nc
    from contextlib import ExitStack
    with ExitStack() as ctx:
        singles = ctx.enter_context(tc.tile_pool(name="singles", bufs=1))
        work = ctx.enter_context(tc.tile_pool(name="work", bufs=4))
        es_pool = ctx.enter_context(tc.tile_pool(name="es_pool", bufs=8))
        dram = ctx.enter_context(tc.tile_pool(name="dram", bufs=1, space="DRAM"))

        # ---- identity (for V transposes) then warmup collective ----------
        ident_f32 = singles.tile([128, 128], F32, tag="ident_f32")
        make_identity(nc, ident_f32)
        ident = singles.tile([128, 128], BF16, tag="ident")
        nc.vector.tensor_copy(ident[:], ident_f32[:])

        warm_in = dram.tile([128, 8], F32, name="warm_in")
        warm_out = dram.tile([N_CORES, 128, 8], F32, addr_space="Shared",
                             name="warm_out")
        nc.gpsimd.collective_compute(
            "AllGather", mybir.AluOpType.bypass,
            replica_groups=[list(range(N_CORES))],
            ins=[warm_in.opt()], outs=[warm_out.opt()])

        # ---- weight DMAs, then x chunks interleaved with pass-A matmuls --
        wq = singles.tile([128, ND, HS], BF16, tag="wq")
        wk = singles.tile([128, ND, HS], BF16, tag="wk")
        wv = singles.tile([128, ND, HS], BF16, tag="wv")
        wo = singles.tile([128, ND, HS], BF16, tag="wo")
        nc.sync.dma_start(out=wq[:], in_=wq_ap.rearrange("(c p) m -> p c m", p=128))
        nc.scalar.dma_start(out=wk[:], in_=wk_ap.rearrange("(c p) m -> p c m", p=128))
        nc.gpsimd.dma_start(out=wv[:], in_=wv_ap.rearrange("(c p) m -> p c m", p=128))

        qT = singles.tile([128, T], BF16, tag="qT")
        kT = singles.tile([128, T], BF16, tag="kT")
        vT = singles.tile([128, T], BF16, tag="vT")
        vn = [singles.tile([128, NK, HD + 1], BF16, tag=f"vn{h}", name=f"vn{h}")
              for h in range(HPC)]
        outT = singles.tile([128, T], BF16, tag="outT")

        ones = singles.tile([128, 1], F32, tag="ones")
        nc.vector.memset(ones[:], 1.0)
        for h in range(HPC):
            nc.vector.tensor_copy(vn[h][:, :, HD:],
                                  ones[:].unsqueeze(1).to_broadcast([128, NK, 1]))

        qdma = [nc.sync, nc.scalar, nc.gpsimd]
        xts = []

        with tc.tile_pool(name="pp", bufs=1, space="PSUM") as pp, \
             tc.tile_pool(name="pt", bufs=2, space="PSUM") as pt:
            # pass A (cols 0..1023): emit each x-chunk DMA right before the
            # matmuls that consume it so the DMA waits stay minimal
            ps_q = pp.tile([128, 2, QW], F32, tag="q", name="ps_q")
            ps_k = pp.tile([128, 2, QW], F32, tag="k", name="ps_k")
            ps_v = pp.tile([128, 2, QW], F32, tag="v", name="ps_v")
            # x-chunk -> queue (x0 split across sync+scalar). x1 goes to
            # sync, not gpsimd: behind wv it would arrive after the d=1
            # matmuls want it (~4us PE stall)
            XQ = [None, 0, 1, 2, 0, 1, 2, 0]
            for d in range(ND):
                xt = singles.tile([128, T], BF16, tag=f"x{d}", name=f"xt{d}")
                if d == 0:
                    # split the first chunk across two queues: the first
                    # matmuls wait on it, so halving its DMA time moves the
                    # whole schedule up
                    nc.sync.dma_start(out=xt[0:64, :], in_=xT_ap[0:64, :])
                    nc.scalar.dma_start(out=xt[64:128, :], in_=xT_ap[64:128, :])
                else:
                    qdma[XQ[d]].dma_start(out=xt[:],
                                          in_=xT_ap[d * 128:(d + 1) * 128, :])
                xts.append(xt)
                f = (d == 0)
                l = (d == ND - 1)
                for ps, w in ((ps_q, wq), (ps_k, wk), (ps_v, wv)):
                    for sub in range(2):
                        cs = slice(sub * QW, (sub + 1) * QW)
                        nc.tensor.matmul(ps[:, sub, :], w[:, d, :],
                                         xts[d][:, cs], start=f, stop=l)
            nc.gpsimd.dma_start(out=wo[:],
                                in_=wo_ap.rearrange("(c p) m -> p c m", p=128))
            nc.vector.tensor_copy(vT[:, 0:2 * QW], ps_v[:])
            nc.vector.tensor_copy(qT[:, 0:2 * QW], ps_q[:])
            nc.vector.tensor_copy(kT[:, 0:2 * QW], ps_k[:])
            # V transposes for blocks 0-7 (PE; waits only on the v copy)
            for tk in range(8):
                ps_t = pt.tile([128, W], BF16, tag="t", name="ps_t")
                nc.tensor.transpose(ps_t[:], vT[:, tk * W:(tk + 1) * W], ident[:])
                for h in range(HPC):
                    nc.vector.tensor_copy(vn[h][:, tk, :HD],
                                          ps_t[:, h * HD:(h + 1) * HD])

            # pass B (cols 1024..2047): all of x is resident by now
            ps_q = pp.tile([128, 2, QW], F32, tag="q", name="ps_qB")
            ps_k = pp.tile([128, 2, QW], F32, tag="k", name="ps_kB")
            ps_v = pp.tile([128, 2, QW], F32, tag="v", name="ps_vB")
            for d in range(ND):
                f = (d == 0)
                l = (d == ND - 1)
                for ps, w in ((ps_q, wq), (ps_k, wk), (ps_v, wv)):
                    for sub in range(2):
                        cs = slice(2 * QW + sub * QW, 2 * QW + (sub + 1) * QW)
                        nc.tensor.matmul(ps[:, sub, :], w[:, d, :],
                                         xts[d][:, cs], start=f, stop=l)
            nc.vector.tensor_copy(vT[:, 2 * QW:], ps_v[:])
            nc.vector.tensor_copy(qT[:, 2 * QW:], ps_q[:])
            nc.vector.tensor_copy(kT[:, 2 * QW:], ps_k[:])
            for tk in range(8, NK):
                ps_t = pt.tile([128, W], BF16, tag="t", name="ps_t")
                nc.tensor.transpose(ps_t[:], vT[:, tk * W:(tk + 1) * W], ident[:])
                for h in range(HPC):
                    nc.vector.tensor_copy(vn[h][:, tk, :HD],
                                          ps_t[:, h * HD:(h + 1) * HD])

        # ---- attention: flat (chunk, keyblock) stream, pipelined exps ----
        # AllGather groups (start_chunk, n_chunks). Each gather has a
        # large fixed cost (~15-20us) regardless of payload, so two equal
        # groups win: {c0,c1} triggers right after c1 (early, absorbing
        # the CC engine's variable init window) and {c2,c3} right after
        # c3; the {c0,c1} output projection covers the second gather.
        PAIRS = [(0, 2), (2, 2)]
        ag_in = [dram.tile([HS, n * QW], BF16, name=f"ag_in{j}")
                 for j, (s, n) in enumerate(PAIRS)]
        ag_out = [dram.tile([N_CORES, HS, n * QW], BF16, addr_space="Shared",
                            name=f"ag_out{j}") for j, (s, n) in enumerate(PAIRS)]

        gt_pool = ctx.enter_context(tc.tile_pool(name="gt_pool", bufs=1))
        gts = {}

        with tc.tile_pool(name="pa", bufs=2, space="PSUM") as pa, \
             tc.tile_pool(name="po", bufs=2, space="PSUM") as po:
            ps_os = {}

            def s_exp(t, tk):
                qs = max(0, (tk - 4 * t) * W)
                ps_s = pa.tile([128, 2 * QW], F32, tag="s", name="ps_s")
                for h in range(HPC):
                    hrows = slice(h * HD, (h + 1) * HD)
                    nc.tensor.matmul(
                        ps_s[:, h * QW + qs:(h + 1) * QW],
                        kT[hrows, tk * W:(tk + 1) * W],
                        qT[hrows, t * QW + qs:(t + 1) * QW],
                        start=True, stop=True)
                e = es_pool.tile([128, 2 * QW], BF16, tag="es", name="es")
                nc.scalar.activation(out=e[:, qs:], in_=ps_s[:, qs:],
                                     func=Exp, scale=SCALE)
                return e

            def av(t, tk, e):
                qs = max(0, (tk - 4 * t) * W)
                n_tk = 4 * t + 4
                if tk == 0:
                    ps_os[t] = po.tile([HD + 1, 2, QW], F32, tag="o",
                                       name="ps_o")
                ps_o = ps_os[t]
                for h in range(HPC):
                    nc.tensor.matmul(ps_o[:, h, qs:], vn[h][:, tk, :],
                                     e[:, h * QW + qs:(h + 1) * QW],
                                     start=(tk == 0), stop=(tk == n_tk - 1))
                if tk == n_tk - 1:
                    _normalize(t, ps_o)

            def _normalize(t, ps_o):
                cols = slice(t * QW, (t + 1) * QW)
                bc = work.tile([HD, 2, QW], F32, tag="bc", name="bc")
                for h in range(HPC):
                    rec = work.tile([1, QW], F32, tag=f"rec{h}", name="rec")
                    nc.vector.reciprocal(out=rec[:], in_=ps_o[HD:, h, :])
                    nc.gpsimd.partition_broadcast(bc[:, h, :], rec[:])
                for h in range(HPC):
                    hrows = slice(h * HD, (h + 1) * HD)
                    nc.vector.tensor_mul(outT[hrows, cols],
                                         ps_o[:HD, h, :], bc[:, h, :])
                jmatch = [j for j, (s, n) in enumerate(PAIRS)
                          if t == s + n - 1]
                if jmatch:
                    j = jmatch[0]
                    s, n = PAIRS[j]
                    pcols = slice(s * QW, (s + n) * QW)
                    nc.gpsimd.dma_start(out=ag_in[j][:], in_=outT[:, pcols])
                    nc.gpsimd.collective_compute(
                        "AllGather", mybir.AluOpType.bypass,
                        replica_groups=[list(range(N_CORES))],
                        ins=[ag_in[j].opt()], outs=[ag_out[j].opt()])
                    # prefetch the gathered slabs. DMA waits are cumulative
                    # per queue, so each group gets its own queue: j0 on
                    # sync (nothing later on sync until its own y store),
                    # j1 on scalar (all exps are done by then) -- the j0
                    # output projection must not wait behind j1's gather.
                    gq = nc.sync if j == 0 else nc.scalar
                    gts[j] = []
                    for c in range(N_CORES):
                        g = gt_pool.tile([128, n * QW], BF16,
                                         tag=f"g{j}_{c}", name=f"g{j}_{c}")
                        gq.dma_start(out=g[:], in_=ag_out[j][c])
                        gts[j].append(g)

            items = [(t, tk) for t in range(NQ) for tk in range(4 * t + 4)]
            pend = []
            for it in items:
                pend.append((it, s_exp(*it)))
                if len(pend) > 2:
                    (pt_, ptk), pe_ = pend.pop(0)
                    av(pt_, ptk, pe_)
            for (pt_, ptk), pe_ in pend:
                av(pt_, ptk, pe_)

        # ---- output projection on gathered activations -------------------
        with tc.tile_pool(name="py", bufs=1, space="PSUM") as py:
            for j, (s, n) in enumerate(PAIRS):
                ps_y = [py.tile([128, QW], F32, tag=f"y{i}", name=f"ps_y{i}")
                        for i in range(n)]
                for c in range(N_CORES):
                    g = gts[j][c]
                    for i in range(n):
                        nc.tensor.matmul(ps_y[i][:], wo[:, c, :],
                                         g[:, i * QW:(i + 1) * QW],
                                         start=(c == 0), stop=(c == N_CORES - 1))
                for i in range(n):
                    t = s + i
                    cols = slice(t * QW, (t + 1) * QW)
                    cy = work.tile([128, QW], F32, tag="cy")
                    nc.vector.tensor_copy(cy[:], ps_y[i][:])
                    nc.sync.dma_start(out=y_ap[:, cols], in_=cy[:])


def make_in_maps(x, Wq, Wk, Wv, Wo):
    bf = ml_dtypes.bfloat16
    xT = np.ascontiguousarray(np.asarray(x).reshape(T, D).T).astype(bf)
    in_maps = []
    for c in range(N_CORES):
        hs = slice(c * HS, (c + 1) * HS)
        in_maps.append({
            "xT": xT,
            "wq": np.ascontiguousarray(np.asarray(Wq)[:, hs]).astype(bf),
            "wk": np.ascontiguousarray(np.asarray(Wk)[:, hs]).astype(bf),
            "wv": np.ascontiguousarray(np.asarray(Wv)[:, hs]).astype(bf),
            "wo": np.ascontiguousarray(np.asarray(Wo)[:, hs]).astype(bf),
        })
    return in_maps


def kernel(x, Wq, Wk, Wv, Wo):
    if "nc" not in _compiled:
        _compiled["nc"] = _build()
    nc = _compiled["nc"]

    in_maps = make_in_maps(x, Wq, Wk, Wv, Wo)
    res = run_bass_kernel_spmd(nc, in_maps, list(range(N_CORES)))
    finalT = np.concatenate([res.results[c]["y"] for c in range(N_CORES)], axis=0)
    return np.ascontiguousarray(finalT.T).reshape(B, T, D)


# revision 27
# speedup vs baseline: 1.0452x; 1.0245x over previous
"""LocalWindowAttention (block-causal) Trainium2 kernel, 8 NeuronCores.

Sharding: tensor-parallel over heads. Core c owns head-columns
[c*128, (c+1)*128) of the D=1024 hidden dim (2 heads x head_dim 64):
  - computes Q/K/V projections for its head slice (transposed layout),
  - block-causal attention for its 2 heads,
  - AllGathers its normalized attention outputs (two chunk-pair AGs),
  - applies the full Wo to the gathered activations for its 128 output
    rows of final^T [1024, 2048]. Host reassembles.

v3:
  - bf16 operands on-chip (PSUM stays f32): 1 cycle/row at any free
    dim, half the DMA of f32.
  - x DMAs are emitted interleaved with the first projection pass's
    matmuls, round-robin over four queues, so each matmul's DMA wait
    only covers chunks emitted before it -> PE starts ~2.5us after the
    queues open instead of after the full x load.
  - tiny warmup AllGather absorbs the CC engine's ~45us cold start.
  - attention runs as one flat (chunk, keyblock) stream with the
    score/exp pipeline carried across chunk boundaries (no drain
    stalls); normalize is emitted inline after each chunk's last AV.
  - two AllGathers (chunks {0,1} after c1, {2,3} after c3); the
    {0,1} output projection covers the second AG's latency.
"""

import numpy as np
import ml_dtypes

import concourse.bacc as bacc
import concourse.tile as tile
from concourse import mybir
from concourse.bass_utils import run_bass_kernel_spmd
from concourse.masks import make_identity

B, T, D = 1, 2048, 1024
H, HD, W = 16, 64, 128
N_CORES = 8
HS = D // N_CORES        # 128 head-columns per core (2 heads)
HPC = H // N_CORES       # heads per core
QW = 512                 # query-chunk width (free dim of S^T tiles)
NQ = T // QW             # 4 query chunks
NK = T // W              # 16 key chunks of 128
ND = D // 128            # 8 contraction chunks over D
SCALE = HD ** -0.5

F32 = mybir.dt.float32
BF16 = mybir.dt.bfloat16
Exp = mybir.ActivationFunctionType.Exp

_compiled = {}


def _build():
    nc = bacc.Bacc("TRN2", target_bir_lowering=False, debug=False,
                   num_devices=N_CORES)
    xT_ap = nc.dram_tensor("xT", [D, T], BF16, kind="ExternalInput").ap()
    wq_ap = nc.dram_tensor("wq", [D, HS], BF16, kind="ExternalInput").ap()
    wk_ap = nc.dram_tensor("wk", [D, HS], BF16, kind="ExternalInput").ap()
    wv_ap = nc.dram_tensor("wv", [D, HS], BF16, kind="ExternalInput").ap()
    wo_ap = nc.dram_tensor("wo", [D, HS], BF16, kind="ExternalInput").ap()
    y_ap = nc.dram_tensor("y", [HS, T], F32, kind="ExternalOutput").ap()

    with tile.TileContext(nc) as tc:
        _body(tc, xT_ap, wq_ap, wk_ap, wv_ap, wo_ap, y_ap)
    nc.compile()
    return nc


def _body(tc, xT_ap, wq_ap, wk_ap, wv_ap, wo_ap, y_ap):
    nc = tc.nc
    from contextlib import ExitStack
    with ExitStack() as ctx:
        singles = ctx.enter_context(tc.tile_pool(name="singles", bufs=1))
        work = ctx.enter_context(tc.tile_pool(name="work", bufs=4))
        es_pool = ctx.enter_context(tc.tile_pool(name="es_pool", bufs=8))
        dram = ctx.enter_context(tc.tile_pool(name="dram", bufs=1, space="DRAM"))

        # ---- identity (for V transposes) then warmup collective ----------
        ident_f32 = singles.tile([128, 128], F32, tag="ident_f32")
        make_identity(nc, ident_f32)
        ident = singles.tile([128, 128], BF16, tag="ident")
        nc.vector.tensor_copy(ident[:], ident_f32[:])

        warm_in = dram.tile([128, 8], F32, name="warm_in")
        warm_out = dram.tile([N_CORES, 128, 8], F32, addr_space="Shared",
                             name="warm_out")
        nc.gpsimd.collective_compute(
            "AllGather", mybir.AluOpType.bypass,
            replica_groups=[list(range(N_CORES))],
            ins=[warm_in.opt()], outs=[warm_out.opt()])

        # ---- weight DMAs, then x chunks interleaved with pass-A matmuls --
        wq = singles.tile([128, ND, HS], BF16, tag="wq")
        wk = singles.tile([128, ND, HS], BF16, tag="wk")
        wv = singles.tile([128, ND, HS], BF16, tag="wv")
        wo = singles.tile([128, ND, HS], BF16, tag="wo")
        nc.sync.dma_start(out=wq[:], in_=wq_ap.rearrange("(c p) m -> p c m", p=128))
        nc.scalar.dma_start(out=wk[:], in_=wk_ap.rearrange("(c p) m -> p c m", p=128))
        nc.gpsimd.dma_start(out=wv[:], in_=wv_ap.rearrange("(c p) m -> p c m", p=128))

        qT = singles.tile([128, T], BF16, tag="qT")
        kT = singles.tile([128, T], BF16, tag="kT")
        vT = singles.tile([128, T], BF16, tag="vT")
        vn = [singles.tile([128, NK, HD + 1], BF16, tag=f"vn{h}", name=f"vn{h}")
              for h in range(HPC)]
        outT = singles.tile([128, T], BF16, tag="outT")

        ones = singles.tile([128, 1], F32, tag="ones")
        nc.vector.memset(ones[:], 1.0)
        for h in range(HPC):
            nc.vector.tensor_copy(vn[h][:, :, HD:],
                                  ones[:].unsqueeze(1).to_broadcast([128, NK, 1]))

        qdma = [nc.sync, nc.scalar, nc.gpsimd]
        xts = []

        with tc.tile_pool(name="pp", bufs=1, space="PSUM") as pp, \
             tc.tile_pool(name="pt", bufs=2, space="PSUM") as pt:
            # pass A (cols 0..1023): emit each x-chunk DMA right before the
            # matmuls that consume it so the DMA waits stay minimal
            ps_q = pp.tile([128, 2, QW], F32, tag="q", name="ps_q")
            ps_k = pp.tile([128, 2, QW], F32, tag="k", name="ps_k")
            ps_v = pp.tile([128, 2, QW], F32, tag="v", name="ps_v")
            # x-chunk -> queue (x0 split across sync+scalar). x1 goes to
            # sync, not gpsimd: behind wv it would arrive after the d=1
            # matmuls want it (~4us PE stall)
            XQ = [None, 0, 1, 2, 0, 1, 2, 0]
            for d in range(ND):
                xt = singles.tile([128, T], BF16, tag=f"x{d}", name=f"xt{d}")
                if d == 0:
                    # split the first chunk across two queues: the first
                    # matmuls wait on it, so halving its DMA time moves the
                    # whole schedule up
                    nc.sync.dma_start(out=xt[0:64, :], in_=xT_ap[0:64, :])
                    nc.scalar.dma_start(out=xt[64:128, :], in_=xT_ap[64:128, :])
                else:
                    qdma[XQ[d]].dma_start(out=xt[:],
                                          in_=xT_ap[d * 128:(d + 1) * 128, :])
                xts.append(xt)
                f = (d == 0)
                l = (d == ND - 1)
                for ps, w in ((ps_q, wq), (ps_k, wk), (ps_v, wv)):
                    for sub in range(2):
                        cs = slice(sub * QW, (sub + 1) * QW)
                        nc.tensor.matmul(ps[:, sub, :], w[:, d, :],
                                         xts[d][:, cs], start=f, stop=l)
            nc.gpsimd.dma_start(out=wo[:],
                                in_=wo_ap.rearrange("(c p) m -> p c m", p=128))
            nc.vector.tensor_copy(vT[:, 0:2 * QW], ps_v[:])
            nc.vector.tensor_copy(qT[:, 0:2 * QW], ps_q[:])
            nc.vector.tensor_copy(kT[:, 0:2 * QW], ps_k[:])
            # V transposes for blocks 0-7 (PE; waits only on the v copy)
            for tk in range(8):
                ps_t = pt.tile([128, W], BF16, tag="t", name="ps_t")
                nc.tensor.transpose(ps_t[:], vT[:, tk * W:(tk + 1) * W], ident[:])
                for h in range(HPC):
                    nc.vector.tensor_copy(vn[h][:, tk, :HD],
                                          ps_t[:, h * HD:(h + 1) * HD])

            # pass B (cols 1024..2047): all of x is resident by now
            ps_q = pp.tile([128, 2, QW], F32, tag="q", name="ps_qB")
            ps_k = pp.tile([128, 2, QW], F32, tag="k", name="ps_kB")
            ps_v = pp.tile([128, 2, QW], F32, tag="v", name="ps_vB")
            for d in range(ND):
                f = (d == 0)
                l = (d == ND - 1)
                for ps, w in ((ps_q, wq), (ps_k, wk), (ps_v, wv)):
                    for sub in range(2):
                        cs = slice(2 * QW + sub * QW, 2 * QW + (sub + 1) * QW)
                        nc.tensor.matmul(ps[:, sub, :], w[:, d, :],
                                         xts[d][:, cs], start=f, stop=l)
            nc.vector.tensor_copy(vT[:, 2 * QW:], ps_v[:])
            nc.vector.tensor_copy(qT[:, 2 * QW:], ps_q[:])
            nc.vector.tensor_copy(kT[:, 2 * QW:], ps_k[:])
            for tk in range(8, NK):
                ps_t = pt.tile([128, W], BF16, tag="t", name="ps_t")
                nc.tensor.transpose(ps_t[:], vT[:, tk * W:(tk + 1) * W], ident[:])
                for h in range(HPC):
                    nc.vector.tensor_copy(vn[h][:, tk, :HD],
                                          ps_t[:, h * HD:(h + 1) * HD])

        # ---- attention: flat (chunk, keyblock) stream, pipelined exps ----
        # AllGather groups (start_chunk, n_chunks). Each gather has a
        # large fixed cost (~15-20us) regardless of payload, so two equal
        # groups win: {c0,c1} triggers right after c1 (early, absorbing
        # the CC engine's variable init window) and {c2,c3} right after
        # c3; the {c0,c1} output projection covers the second gather.
        PAIRS = [(0, 2), (2, 2)]
        ag_in = [dram.tile([HS, n * QW], BF16, name=f"ag_in{j}")
                 for j, (s, n) in enumerate(PAIRS)]
        ag_out = [dram.tile([N_CORES, HS, n * QW], BF16, addr_space="Shared",
                            name=f"ag_out{j}") for j, (s, n) in enumerate(PAIRS)]

        gt_pool = ctx.enter_context(tc.tile_pool(name="gt_pool", bufs=1))
        gts = {}

        with tc.tile_pool(name="pa", bufs=2, space="PSUM") as pa, \
             tc.tile_pool(name="po", bufs=2, space="PSUM") as po:
            ps_os = {}

            def s_exp(t, tk):
                qs = max(0, (tk - 4 * t) * W)
                ps_s = pa.tile([128, 2 * QW], F32, tag="s", name="ps_s")
                for h in range(HPC):
                    hrows = slice(h * HD, (h + 1) * HD)
                    nc.tensor.matmul(
                        ps_s[:, h * QW + qs:(h + 1) * QW],
                        kT[hrows, tk * W:(tk + 1) * W],
                        qT[hrows, t * QW + qs:(t + 1) * QW],
                        start=True, stop=True)
                e = es_pool.tile([128, 2 * QW], BF16, tag="es", name="es")
                nc.scalar.activation(out=e[:, qs:], in_=ps_s[:, qs:],
                                     func=Exp, scale=SCALE)
                return e

            def av(t, tk, e):
                qs = max(0, (tk - 4 * t) * W)
                n_tk = 4 * t + 4
                if tk == 0:
                    ps_os[t] = po.tile([HD + 1, 2, QW], F32, tag="o",
                                       name="ps_o")
                ps_o = ps_os[t]
                for h in range(HPC):
                    nc.tensor.matmul(ps_o[:, h, qs:], vn[h][:, tk, :],
                                     e[:, h * QW + qs:(h + 1) * QW],
                                     start=(tk == 0), stop=(tk == n_tk - 1))
                if tk == n_tk - 1:
                    _normalize(t, ps_o)

            def _normalize(t, ps_o):
                cols = slice(t * QW, (t + 1) * QW)
                bc = work.tile([HD, 2, QW], F32, tag="bc", name="bc")
                for h in range(HPC):
                    rec = work.tile([1, QW], F32, tag=f"rec{h}", name="rec")
                    nc.vector.reciprocal(out=rec[:], in_=ps_o[HD:, h, :])
                    nc.gpsimd.partition_broadcast(bc[:, h, :], rec[:])
                for h in range(HPC):
                    hrows = slice(h * HD, (h + 1) * HD)
                    nc.vector.tensor_mul(outT[hrows, cols],
                                         ps_o[:HD, h, :], bc[:, h, :])
                jmatch = [j for j, (s, n) in enumerate(PAIRS)
                          if t == s + n - 1]
                if jmatch:
                    j = jmatch[0]
                    s, n = PAIRS[j]
                    pcols = slice(s * QW, (s + n) * QW)
                    nc.gpsimd.dma_start(out=ag_in[j][:], in_=outT[:, pcols])
                    nc.gpsimd.collective_compute(
                        "AllGather", mybir.AluOpType.bypass,
                        replica_groups=[list(range(N_CORES))],
                        ins=[ag_in[j].opt()], outs=[ag_out[j].opt()])
                    # prefetch the gathered slabs. DMA waits are cumulative
                    # per queue, so each group gets its own queue: j0 on
                    # sync (nothing later on sync until its own y store),
                    # j1 on scalar (all exps are done by then) -- the j0
                    # output projection must not wait behind j1's gather.
                    gq = nc.sync if j == 0 else nc.scalar
                    gts[j] = []
                    for c in range(N_CORES):
                        g = gt_pool.tile([128, n * QW], BF16,
                                         tag=f"g{j}_{c}", name=f"g{j}_{c}")
                        gq.dma_start(out=g[:], in_=ag_out[j][c])
                        gts[j].append(g)

            items = [(t, tk) for t in range(NQ) for tk in range(4 * t + 4)]
            pend = []
            for it in items:
                pend.append((it, s_exp(*it)))
                if len(pend) > 2:
                    (pt_, ptk), pe_ = pend.pop(0)
                    av(pt_, ptk, pe_)
            for (pt_, ptk), pe_ in pend:
                av(pt_, ptk, pe_)

        # ---- output projection on gathered activations -------------------
        with tc.tile_pool(name="py", bufs=1, space="PSUM") as py:
            for j, (s, n) in enumerate(PAIRS):
                ps_y = [py.tile([128, QW], F32, tag=f"y{i}", name=f"ps_y{i}")
                        for i in range(n)]
                for c in range(N_CORES):
                    g = gts[j][c]
                    for i in range(n):
                        nc.tensor.matmul(ps_y[i][:], wo[:, c, :],
                                         g[:, i * QW:(i + 1) * QW],
                                         start=(c == 0), stop=(c == N_CORES - 1))
                for i in range(n):
                    t = s + i
                    cols = slice(t * QW, (t + 1) * QW)
                    cy = work.tile([128, QW], F32, tag="cy")
                    nc.vector.tensor_copy(cy[:], ps_y[i][:])
                    nc.sync.dma_start(out=y_ap[:, cols], in_=cy[:])


def make_in_maps(x, Wq, Wk, Wv, Wo):
    bf = ml_dtypes.bfloat16
    xT = np.ascontiguousarray(np.asarray(x).reshape(T, D).T).astype(bf)
    in_maps = []
    for c in range(N_CORES):
        hs = slice(c * HS, (c + 1) * HS)
        in_maps.append({
            "xT": xT,
            "wq": np.ascontiguousarray(np.asarray(Wq)[:, hs]).astype(bf),
            "wk": np.ascontiguousarray(np.asarray(Wk)[:, hs]).astype(bf),
            "wv": np.ascontiguousarray(np.asarray(Wv)[:, hs]).astype(bf),
            "wo": np.ascontiguousarray(np.asarray(Wo)[:, hs]).astype(bf),
        })
    return in_maps


def kernel(x, Wq, Wk, Wv, Wo):
    if "nc" not in _compiled:
        _compiled["nc"] = _build()
    nc = _compiled["nc"]

    in_maps = make_in_maps(x, Wq, Wk, Wv, Wo)
    res = run_bass_kernel_spmd(nc, in_maps, list(range(N_CORES)))
    finalT = np.concatenate([res.results[c]["y"] for c in range(N_CORES)], axis=0)
    return np.ascontiguousarray(finalT.T).reshape(B, T, D)
